# revision 19
# baseline (speedup 1.0000x reference)
"""AttentionBlock Trainium2 Bass kernel (fp8 DoubleRow redesign).

Data-parallel over batch: 16 batches / 8 cores = 2 per core.

Key design points vs the fp32r baseline:
- fp8e4m3 DoubleRow matmuls (contraction 256/instr, 0.5 cyc/row) for the
  q/k/v projections, QK scores, and AV; fp32r only for the output
  projection (accuracy).
- Weight pre-scaling: Wq,Wk x16 (compensated by exp scale 1/256), Wv x16
  (compensated by Wp/16) keeps fp8 weight entries out of the subnormal
  range.
- QK packs 4 heads' k-channels (256) into one DoubleRow contraction with
  per-head zero-padded q operands (zeros memset once, rows rewritten per
  batch).
- exp split across engines: Act does native exp -> fp8 wts; DVE/Pool do
  Schraudolph bf16 exp (int16 bit trick) for a subset of seq-chunks whose
  AV runs in bf16.
- softmax denominator from a ones-column appended to vT (row 64 of the AV
  psum); reciprocal on DVE, broadcast via a tiny PE matmul (ones x recip)
  into PSUM, normalize fused with the PSUM->SBUF move.
- groupnorm stats via bn_stats on bf16 inputs + block-diag matmul
  reduction; rsqrt via quake seed + 2 Newton steps (keeps Act on exp).
- x/y staged in bf16 (halves DMA); residual add reuses the staged x.
"""
import os
import sys

sys.path.insert(0, "/opt/trn_rl_repo")

import numpy as np

import concourse.bacc as bacc
import concourse.bass as bass
import concourse.tile as tile
from concourse import mybir
from concourse.bass_utils import run_bass_kernel_spmd

F32 = mybir.dt.float32
F32R = mybir.dt.float32r
BF16 = mybir.dt.bfloat16
FP8 = mybir.dt.float8e4
I16 = mybir.dt.int16
I32 = mybir.dt.int32
AF = mybir.ActivationFunctionType
OP = mybir.AluOpType
PM = mybir.MatmulPerfMode

B, C, H, W = 16, 512, 32, 32
T = H * W              # 1024
NH = 8                 # heads
CH = C // NH           # 64
GROUPS = 32
GSIZE = C // GROUPS    # 16
EPS = 1e-5
N_CORES = 8
BPC = B // N_CORES     # batches per core
CB = C // 128          # 4 channel blocks
NT = T // 512          # 2 column halves
ST = T // 128          # 8 seq tiles of 128
NK = ST // 2           # 4 DoubleRow seq chunks of 256

WSCALE = 16.0          # fp8 weight pre-scale for Wq/Wk/Wv
QK_SCALE = 1.0 / (WSCALE * WSCALE)   # folded into exp
# Schraudolph bf16 exp: bits16 = x*(2^7/ln2)*QK_SCALE + (127*2^7 - c)
SCH_A = 184.66496580927726 * QK_SCALE
SCH_B = 16250.4

# exp engine per k-chunk (0..3): 'A' = Act native exp (fp8 wts, DR AV),
# 'D'/'P' = DVE/Pool Schraudolph (bf16 wts, bf16 AV). Spill chunks first so
# the head tail drains through the fast Act consumer.
EXP_ENG = ['P', 'D', 'A', 'A']


def _build():
    nc = bacc.Bacc(None, target_bir_lowering=False)

    x2 = nc.dram_tensor("x2", (BPC, C, T), BF16, kind="ExternalInput")
    y2 = nc.dram_tensor("y2", (BPC, C, T), BF16, kind="ExternalInput")
    wq8_d = nc.dram_tensor("wq8", (128, 2, 2, C), FP8, kind="ExternalInput")
    wk8_d = nc.dram_tensor("wk8", (128, 2, 2, C), FP8, kind="ExternalInput")
    wv8_d = nc.dram_tensor("wv8", (128, 2, 2, C), FP8, kind="ExternalInput")
    wpt = nc.dram_tensor("wpt", (C, C), F32, kind="ExternalInput")
    bq_l = nc.dram_tensor("bq_l", (128, CB), F32, kind="ExternalInput")
    bk_l = nc.dram_tensor("bk_l", (128, CB), F32, kind="ExternalInput")
    bp_l = nc.dram_tensor("bp_l", (128, CB), F32, kind="ExternalInput")
    bv_bc = nc.dram_tensor("bv_bc", (128, NH, CH), F32, kind="ExternalInput")
    gnw_l = nc.dram_tensor("gnw_l", (128, CB), F32, kind="ExternalInput")
    gnb_l = nc.dram_tensor("gnb_l", (128, CB), F32, kind="ExternalInput")
    m1 = nc.dram_tensor("m1", (128, 128), F32, kind="ExternalInput")
    vcap8_d = nc.dram_tensor("vcap8", (128, NH, 2), FP8, kind="ExternalInput")
    vcap16_d = nc.dram_tensor("vcap16", (128, NH, 2), BF16, kind="ExternalInput")
    ones64_d = nc.dram_tensor("ones64", (1, CH), F32, kind="ExternalInput")
    out_d = nc.dram_tensor("out", (BPC, C, T), F32, kind="ExternalOutput")

    with tile.TileContext(nc) as tc:
        from contextlib import ExitStack
        with ExitStack() as ctx:
            consts = ctx.enter_context(tc.tile_pool(name="consts", bufs=1))
            px = ctx.enter_context(tc.tile_pool(name="px", bufs=2))
            py = ctx.enter_context(tc.tile_pool(name="py", bufs=2))
            pgn = ctx.enter_context(tc.tile_pool(name="pgn", bufs=4))
            pkq = ctx.enter_context(tc.tile_pool(name="pkq", bufs=4))
            pvt8 = ctx.enter_context(tc.tile_pool(name="pvt8", bufs=4))
            pvt16 = ctx.enter_context(tc.tile_pool(name="pvt16", bufs=8))
            pw8 = ctx.enter_context(tc.tile_pool(name="pw8", bufs=6))
            pw16 = ctx.enter_context(tc.tile_pool(name="pw16", bufs=4))
            pa = ctx.enter_context(tc.tile_pool(name="pa", bufs=2))
            pr0 = ctx.enter_context(tc.tile_pool(name="pr0", bufs=2))
            pst = ctx.enter_context(tc.tile_pool(name="pst", bufs=4))
            post = ctx.enter_context(tc.tile_pool(name="post", bufs=4))
            ps_proj = ctx.enter_context(tc.tile_pool(name="ps_proj", bufs=1, space="PSUM"))
            ps_sc = ctx.enter_context(tc.tile_pool(name="ps_sc", bufs=2, space="PSUM"))
            ps_a = ctx.enter_context(tc.tile_pool(name="ps_a", bufs=1, space="PSUM"))
            ps_aux = ctx.enter_context(tc.tile_pool(name="ps_aux", bufs=1, space="PSUM"))

            # ---------------- constants ----------------
            wq_sb = consts.tile([128, 2, 2, C], FP8, tag="wq")
            wk_sb = consts.tile([128, 2, 2, C], FP8, tag="wk")
            wv_sb = consts.tile([128, 2, 2, C], FP8, tag="wv")
            wp_sb = consts.tile([128, CB, C], F32R, tag="wp")

            def emit_weight_loads():
                nc.sync.dma_start(out=wk_sb, in_=wk8_d[:, :, :, :])
                nc.sync.dma_start(out=wv_sb, in_=wv8_d[:, :, :, :])
                nc.sync.dma_start(out=wq_sb, in_=wq8_d[:, :, :, :])
                nc.sync.dma_start(out=wp_sb, in_=wpt.rearrange("(kb p) o -> p kb o", p=128).bitcast(F32R))

            m1_sb = consts.tile([128, 128], F32, tag="m1")
            bq_sb = consts.tile([128, CB], F32, tag="bq")
            bk_sb = consts.tile([128, CB], F32, tag="bk")
            bp_sb = consts.tile([128, CB], F32, tag="bp")
            bv_sb = consts.tile([128, NH, CH], F32, tag="bv")
            gnw_sb = consts.tile([128, CB], F32, tag="gnw")
            gnb_sb = consts.tile([128, CB], F32, tag="gnb")
            vcap8_sb = consts.tile([128, NH, 2], FP8, tag="vcap8")
            vcap16_sb = consts.tile([128, NH, 2], BF16, tag="vcap16")
            ones64_sb = consts.tile([1, CH], F32, tag="ones64")
            magic_sb = consts.tile([128, CB], I32, tag="magic")
            nc.vector.memset(magic_sb, 0x5f3759df)
            warm = consts.tile([1, 1], F32, tag="warm")
            nc.vector.memset(warm, 0.0)
            nc.scalar.activation(out=warm, in_=warm, func=AF.Exp)

            def emit_small_consts():
                nc.sync.dma_start(out=m1_sb, in_=m1[:, :])
                nc.sync.dma_start(out=gnw_sb, in_=gnw_l[:, :])
                nc.sync.dma_start(out=gnb_sb, in_=gnb_l[:, :])
                nc.sync.dma_start(out=bk_sb, in_=bk_l[:, :])
                nc.sync.dma_start(out=bq_sb, in_=bq_l[:, :])
                nc.sync.dma_start(out=bv_sb, in_=bv_bc[:, :, :])
                nc.sync.dma_start(out=bp_sb, in_=bp_l[:, :])
                nc.sync.dma_start(out=vcap8_sb, in_=vcap8_d[:, :, :])
                nc.sync.dma_start(out=vcap16_sb, in_=vcap16_d[:, :, :])
                nc.sync.dma_start(out=ones64_sb, in_=ones64_d[:, :])

            # persistent zero-padded q operands: [128, 2, T] fp8 per head;
            # head h occupies rows (h%2)*64.. at sub (h//2)%2; zeros persist.
            q_pads = [consts.tile([128, 2, T], FP8, tag=f"qpad{h}", name=f"qpad{h}")
                      for h in range(NH)]

            def emit_qpad_memsets():
                for h, qp in enumerate(q_pads):
                    eng = (nc.gpsimd, nc.vector)[h % 2]
                    eng.memset(qp.rearrange("p a b -> p (a b)"), 0.0)

            # ---------------- groupnorm ----------------
            def groupnorm(src_sb, gn_tiles, spread=False):
                """src_sb: [128, CB, T] bf16. gn_tiles: 2 x [128, 2, T] fp8."""
                mv = pst.tile([128, CB, 2], F32, tag="mv")
                stats6 = pst.tile([128, 2, 6], F32, tag="stats6")
                for cb in range(CB):
                    for c2 in range(2):
                        nc.vector.bn_stats(
                            out=stats6[:, c2, :],
                            in_=src_sb[:, cb, c2 * 512:(c2 + 1) * 512])
                    nc.vector.bn_aggr(out=mv[:, cb, :], in_=stats6)
                musq = pst.tile([128, 4], F32, tag="musq")
                nc.vector.tensor_tensor(out=musq, in0=mv[:, :, 0], in1=mv[:, :, 0], op=OP.mult)
                nc.vector.tensor_tensor(out=mv[:, :, 1], in0=musq, in1=mv[:, :, 1], op=OP.add)
                aux = ps_aux.tile([128, 512], F32, tag="aux")
                psg = aux[:, 0:8]
                nc.tensor.matmul(psg, m1_sb, mv.rearrange("p a b -> p (a b)"), start=True, stop=True)
                gsb = pst.tile([128, 8], F32, tag="gsb")
                nc.vector.tensor_copy(gsb, psg)
                tmp4 = pst.tile([128, 4], F32, tag="tmp4")
                nc.vector.tensor_tensor(out=tmp4, in0=gsb[:, 0::2], in1=gsb[:, 0::2], op=OP.mult)
                vv = pst.tile([128, 4], F32, tag="vv")
                nc.vector.scalar_tensor_tensor(
                    out=vv, in0=gsb[:, 1::2], scalar=EPS, in1=tmp4,
                    op0=OP.add, op1=OP.subtract)
                bsh = pst.tile([128, 4], I32, tag="bsh")
                nc.vector.tensor_scalar(
                    out=bsh, in0=vv.bitcast(I32), scalar1=1, scalar2=None,
                    op0=OP.logical_shift_right)
                nc.vector.tensor_tensor(out=tmp4.bitcast(I32), in0=magic_sb, in1=bsh, op=OP.subtract)
                nrt = pst.tile([128, 4], F32, tag="nrt")
                for _ in range(2):
                    nc.vector.tensor_tensor(out=nrt, in0=tmp4, in1=tmp4, op=OP.mult)
                    nc.vector.scalar_tensor_tensor(
                        out=nrt, in0=nrt, scalar=-0.5, in1=vv, op0=OP.mult, op1=OP.mult)
                    nc.vector.scalar_tensor_tensor(
                        out=tmp4, in0=nrt, scalar=1.5, in1=tmp4, op0=OP.add, op1=OP.mult)
                ab = pst.tile([128, 8], F32, tag="ab")
                nc.vector.tensor_tensor(out=ab[:, 0:4], in0=tmp4, in1=gnw_sb, op=OP.mult)
                tmp4b = pst.tile([128, 4], F32, tag="tmp4b")
                nc.vector.tensor_tensor(out=tmp4b, in0=gsb[:, 0::2], in1=ab[:, 0:4], op=OP.mult)
                nc.vector.tensor_tensor(out=ab[:, 4:8], in0=gnb_sb, in1=tmp4b, op=OP.subtract)
                for cb in range(CB):
                    kb2, i = cb // 2, cb % 2
                    for th in range(NT):
                        eng = nc.vector if not spread else \
                            (nc.vector, nc.gpsimd)[(cb * NT + th) % 2]
                        eng.tensor_scalar(
                            out=gn_tiles[kb2][:, i, th * 512:(th + 1) * 512],
                            in0=src_sb[:, cb, th * 512:(th + 1) * 512],
                            scalar1=ab[:, cb:cb + 1], scalar2=ab[:, 4 + cb:5 + cb],
                            op0=OP.mult, op1=OP.add)

            def emit_load(b, dram, pool, tag):
                sb = pool.tile([128, CB, T], BF16, tag=tag, name=tag)
                for cb in range(CB):
                    nc.sync.dma_start(
                        out=sb[:, cb, :],
                        in_=dram[b].rearrange("(cb p) t -> p cb t", p=128)[:, cb, :])
                return sb

            def emit_loads(b):
                y_sb = emit_load(b, y2, py, "y")
                x_sb = emit_load(b, x2, px, "x")
                return x_sb, y_sb

            def emit_gn(src_sb, tag, spread=False):
                gn_tiles = [pgn.tile([128, 2, T], FP8, tag=tag, name=f"{tag}{kb2}")
                            for kb2 in range(2)]
                groupnorm(src_sb, gn_tiles, spread=spread)
                return gn_tiles

            # ---------------- projections ----------------
            def k_proj(gny):
                kq = [pkq.tile([128, 2, T], FP8, tag="kq", name=f"kq{q}") for q in range(2)]
                for ob in range(CB):
                    quad, i = ob // 2, ob % 2
                    for th in range(NT):
                        psk = ps_proj.tile([128, 512], F32, tag="mm")
                        for kb2 in range(2):
                            nc.tensor.matmul(
                                psk,
                                wk_sb[:, kb2, :, ob * 128:(ob + 1) * 128],
                                gny[kb2][:, :, th * 512:(th + 1) * 512],
                                start=(kb2 == 0), stop=(kb2 == 1),
                                perf_mode=PM.DoubleRow)
                        nc.scalar.activation(
                            out=kq[quad][:, i, th * 512:(th + 1) * 512],
                            in_=psk, func=AF.Identity,
                            bias=bk_sb[:, ob:ob + 1], scale=1.0)
                return kq

            def q_proj(gnx):
                for ob in range(CB):
                    sub = ob % 2
                    h0, h1 = 2 * ob, 2 * ob + 1
                    for th in range(NT):
                        psq = ps_proj.tile([128, 512], F32, tag="mm")
                        for kb2 in range(2):
                            nc.tensor.matmul(
                                psq,
                                wq_sb[:, kb2, :, ob * 128:(ob + 1) * 128],
                                gnx[kb2][:, :, th * 512:(th + 1) * 512],
                                start=(kb2 == 0), stop=(kb2 == 1),
                                perf_mode=PM.DoubleRow)
                        nc.scalar.activation(
                            out=q_pads[h0][0:64, sub, th * 512:(th + 1) * 512],
                            in_=psq[0:64, :], func=AF.Identity,
                            bias=bq_sb[0:64, ob:ob + 1], scale=1.0)
                        nc.scalar.activation(
                            out=q_pads[h1][64:128, sub, th * 512:(th + 1) * 512],
                            in_=psq[64:128, :], func=AF.Identity,
                            bias=bq_sb[64:128, ob:ob + 1], scale=1.0)

            def v_proj(gny):
                vt8 = {k: pvt8.tile([128, 2, NH, CH + 2], FP8, tag="vt8", name=f"vt8_{k}")
                       for k in range(NK) if EXP_ENG[k] == 'A'}
                vt16 = {st: pvt16.tile([128, NH, CH + 2], BF16, tag="vt16", name=f"vt16_{st}")
                        for st in range(ST) if EXP_ENG[st // 2] != 'A'}
                for tt in range(ST):
                    psv = ps_proj.tile([128, 512], F32, tag="mm")
                    for kb2 in range(2):
                        nc.tensor.matmul(
                            psv,
                            gny[kb2][:, :, tt * 128:(tt + 1) * 128],
                            wv_sb[:, kb2, :, :],
                            start=(kb2 == 0), stop=(kb2 == 1),
                            perf_mode=PM.DoubleRow)
                    if EXP_ENG[tt // 2] == 'A':
                        dst = vt8[tt // 2]
                        nc.vector.tensor_tensor(
                            out=dst[:, tt % 2, :, 0:CH],
                            in0=psv.rearrange("p (h c) -> p h c", h=NH),
                            in1=bv_sb, op=OP.add)
                        nc.vector.tensor_copy(dst[:, tt % 2, :, CH:CH + 2], vcap8_sb)
                    else:
                        dst = vt16[tt]
                        nc.vector.tensor_tensor(
                            out=dst[:, :, 0:CH],
                            in0=psv.rearrange("p (h c) -> p h c", h=NH),
                            in1=bv_sb, op=OP.add)
                        nc.vector.tensor_copy(dst[:, :, CH:CH + 2], vcap16_sb)
                return vt8, vt16

            # ---------------- attention ----------------
            def attention_head(bctx, b, h, a_sb):
                kq = bctx["kq"]
                vt8, vt16 = bctx["vt"]
                quad = h // 4
                psa = ps_a.tile([CH + 2, T], F32, tag="psa")
                last_k = NK - 1
                for k in range(NK):
                    eng = EXP_ENG[k]
                    if eng == 'A':
                        wts = pw8.tile([128, 2, T], FP8, tag="w8", name=f"w8_{h}_{k}")
                    else:
                        wts = pw16.tile([128, 2, T], BF16, tag="w16", name=f"w16_{h}_{k}")
                    for stp in range(2):
                        st = 2 * k + stp
                        scores = ps_sc.tile([128, T], F32, tag="sc")
                        for th in range(NT):
                            nc.tensor.matmul(
                                scores[:, th * 512:(th + 1) * 512],
                                kq[quad][:, :, st * 128:(st + 1) * 128],
                                q_pads[h][:, :, th * 512:(th + 1) * 512],
                                start=True, stop=True,
                                perf_mode=PM.DoubleRow)
                        if eng == 'A':
                            nc.scalar.activation(
                                out=wts[:, stp, :], in_=scores,
                                func=AF.Exp, scale=QK_SCALE)
                        else:
                            veng = nc.vector if eng == 'D' else nc.gpsimd
                            veng.tensor_scalar(
                                out=wts[:, stp, :].bitcast(I16),
                                in0=scores, scalar1=SCH_A, scalar2=SCH_B,
                                op0=OP.mult, op1=OP.add)
                    # AV for chunk k (accumulate into psa)
                    if eng == 'A':
                        for th in range(NT):
                            nc.tensor.matmul(
                                psa[:, th * 512:(th + 1) * 512],
                                vt8[k][:, :, h, :],
                                wts[:, :, th * 512:(th + 1) * 512],
                                start=(k == 0), stop=(k == last_k),
                                perf_mode=PM.DoubleRow,
                                skip_group_check=True)
                    else:
                        for stp in range(2):
                            st = 2 * k + stp
                            for th in range(NT):
                                nc.tensor.matmul(
                                    psa[:, th * 512:(th + 1) * 512],
                                    vt16[st][:, h, :],
                                    wts[:, stp, th * 512:(th + 1) * 512],
                                    start=(k == 0 and stp == 0),
                                    stop=(k == last_k and stp == 1),
                                    skip_group_check=True)
                # ---- softmax denominator + normalize into a_sb ----
                rows = slice((h % 2) * 64, (h % 2) * 64 + 64)
                cbh = h // 2
                r0r = pr0.tile([1, T], F32, tag="r0")
                for th in range(NT):
                    nc.vector.reciprocal_approx_fast(
                        out=r0r[:, th * 512:(th + 1) * 512],
                        in_=psa[CH:CH + 1, th * 512:(th + 1) * 512])
                    aux = ps_aux.tile([128, 512], F32, tag="aux")
                    psb = aux[0:64, :]
                    nc.tensor.matmul(
                        psb, ones64_sb.bitcast(F32R),
                        r0r[:, th * 512:(th + 1) * 512].bitcast(F32R),
                        start=True, stop=True)
                    eng = nc.vector if th == 0 else nc.gpsimd
                    eng.tensor_tensor(
                        out=a_sb[rows, cbh, th * 512:(th + 1) * 512],
                        in0=psa[0:CH, th * 512:(th + 1) * 512],
                        in1=psb, op=OP.mult)

            # ---------------- output projection ----------------
            def p_proj_ob(b, a_sb, x_sb, ob):
                for th in range(NT):
                    psh = ps_proj.tile([128, 512], F32, tag="mm")
                    for kb in range(CB):
                        nc.tensor.matmul(
                            psh,
                            wp_sb[:, kb, ob * 128:(ob + 1) * 128],
                            a_sb[:, kb, th * 512:(th + 1) * 512],
                            start=(kb == 0), stop=(kb == CB - 1))
                    ost = post.tile([128, 512], F32, tag="ost")
                    eng = nc.gpsimd if th % 2 == 0 else nc.vector
                    eng.scalar_tensor_tensor(
                        out=ost, in0=psh, scalar=bp_sb[:, ob:ob + 1],
                        in1=x_sb[:, ob, th * 512:(th + 1) * 512],
                        op0=OP.add, op1=OP.add)
                    nc.sync.dma_start(
                        out=out_d[b].rearrange("(cb p) t -> p cb t", p=128)[:, ob, th * 512:(th + 1) * 512],
                        in_=ost)

            # ---------------- batch pipeline ----------------
            bctxs = [dict() for _ in range(BPC)]
            y0 = emit_load(0, y2, py, "y")
            emit_small_consts()
            x0 = emit_load(0, x2, px, "x")
            bctxs[0]["x"] = x0
            emit_weight_loads()
            with tc.high_priority(10**6):
                bctxs[0]["gny"] = emit_gn(y0, "gny", spread=True)
                bctxs[0]["gnx"] = emit_gn(x0, "gnx", spread=True)
            emit_qpad_memsets()
            with tc.high_priority(10**6):
                bctxs[0]["kq"] = k_proj(bctxs[0]["gny"])
                bctxs[0]["vt"] = v_proj(bctxs[0]["gny"])
                q_proj(bctxs[0]["gnx"])

            prev = None  # (b, a_sb, x_sb) of the previous batch, p-proj pending
            for b in range(BPC):
                bctx = bctxs[b]
                nb = bctxs[b + 1] if b + 1 < BPC else None
                a_sb = pa.tile([128, CB, T], F32R, tag="a")
                for h in range(NH):
                    # attention stream outranks injected background work in
                    # the scheduler's priority heap (relative order preserved)
                    with tc.high_priority(10**6):
                        attention_head(bctx, b, h, a_sb)
                    if prev is not None and h < CB:
                        p_proj_ob(prev[0], prev[1], prev[2], h)
                        if h == CB - 1:
                            prev = None
                    if nb is not None:
                        if h == 0:
                            nxy = emit_loads(b + 1)
                            nb["x"] = nxy[0]
                            nb["_y"] = nxy[1]
                        elif h == 1:
                            nb["gny"] = emit_gn(nb["_y"], "gny")
                        elif h == 3:
                            nb["gnx"] = emit_gn(nb["x"], "gnx")
                        elif h == 5:
                            nb["kq"] = k_proj(nb["gny"])
                        elif h == 6:
                            nb["vt"] = v_proj(nb["gny"])
                        elif h == 7:
                            q_proj(nb["gnx"])
                prev = (b, a_sb, bctx["x"])
            for ob in range(CB):
                p_proj_ob(prev[0], prev[1], prev[2], ob)

    nc.finalize()
    return nc


_NC = None


def _get_nc():
    global _NC
    if _NC is None:
        _NC = _build()
    return _NC


def _prep_inputs(x, y, gn_w, gn_b, Wq, bq, Wkv, bkv, Wp, bp):
    import ml_dtypes
    FP8NP = ml_dtypes.float8_e4m3fn
    scale = CH ** -0.25
    idx_k = np.concatenate([np.arange(h * 2 * CH, h * 2 * CH + CH) for h in range(NH)])
    idx_v = np.concatenate([np.arange(h * 2 * CH + CH, (h + 1) * 2 * CH) for h in range(NH)])

    def dr_layout(wt):  # [C_in, C_out] -> [128, 2, 2, C_out]
        return np.ascontiguousarray(wt.reshape(2, 2, 128, C).transpose(2, 0, 1, 3))

    wq8 = dr_layout((Wq * (scale * WSCALE)).T).astype(FP8NP)
    wk8 = dr_layout((Wkv[idx_k] * (scale * WSCALE)).T).astype(FP8NP)
    wv8 = dr_layout((Wkv[idx_v] * WSCALE).T).astype(FP8NP)
    wpt = np.ascontiguousarray(Wp.T / WSCALE).astype(np.float32)

    def part_layout(v):
        return np.ascontiguousarray(v.reshape(CB, 128).T)

    bq_l = part_layout(bq * (scale * WSCALE))
    bk_l = part_layout(bkv[idx_k] * (scale * WSCALE))
    bp_l = part_layout(bp)
    gnw_l = part_layout(gn_w)
    gnb_l = part_layout(gn_b)
    bv = bkv[idx_v] * WSCALE
    bv_bc = np.broadcast_to(bv.reshape(1, NH, CH), (128, NH, CH)).copy().astype(np.float32)
    m1 = np.zeros((128, 128), np.float32)
    for g in range(128 // GSIZE):
        m1[g * GSIZE:(g + 1) * GSIZE, g * GSIZE:(g + 1) * GSIZE] = 1.0 / GSIZE
    vcap = np.zeros((128, NH, 2), np.float32)
    vcap[:, :, 0] = 1.0
    vcap8 = vcap.astype(FP8NP)
    vcap16 = vcap.astype(ml_dtypes.bfloat16)
    ones64 = np.ones((1, CH), np.float32)

    xf = x.reshape(B, C, T).astype(ml_dtypes.bfloat16)
    yf = y.reshape(B, C, T).astype(ml_dtypes.bfloat16)

    shared = {
        "wq8": wq8, "wk8": wk8, "wv8": wv8, "wpt": wpt,
        "bq_l": bq_l, "bk_l": bk_l, "bp_l": bp_l, "bv_bc": bv_bc,
        "gnw_l": gnw_l, "gnb_l": gnb_l, "m1": m1,
        "vcap8": vcap8, "vcap16": vcap16, "ones64": ones64,
    }
    in_maps = []
    for i in range(N_CORES):
        m = dict(shared)
        m["x2"] = np.ascontiguousarray(xf[i * BPC:(i + 1) * BPC])
        m["y2"] = np.ascontiguousarray(yf[i * BPC:(i + 1) * BPC])
        in_maps.append(m)
    return in_maps


def kernel(x, y, gn_w, gn_b, Wq, bq, Wkv, bkv, Wp, bp):
    args = [np.asarray(a, dtype=np.float32) for a in
            (x, y, gn_w, gn_b, Wq, bq, Wkv, bkv, Wp, bp)]
    in_maps = _prep_inputs(*args)
    nc = _get_nc()
    res = run_bass_kernel_spmd(nc, in_maps, core_ids=list(range(N_CORES)))
    out = np.empty((B, C, T), np.float32)
    for i in range(N_CORES):
        out[i * BPC:(i + 1) * BPC] = res.results[i]["out"]
    return out.reshape(B, C, H, W)


# revision 22
# speedup vs baseline: 1.1251x; 1.1251x over previous
"""AttentionBlock Trainium2 Bass kernel (fp8 DoubleRow redesign).

Data-parallel over batch: 16 batches / 8 cores = 2 per core.

Key design points vs the fp32r baseline:
- fp8e4m3 DoubleRow matmuls (contraction 256/instr, 0.5 cyc/row) for the
  q/k/v projections, QK scores, and AV; fp32r only for the output
  projection (accuracy).
- Weight pre-scaling: Wq,Wk x16 (compensated by exp scale 1/256), Wv x16
  (compensated by Wp/16) keeps fp8 weight entries out of the subnormal
  range.
- QK packs 4 heads' k-channels (256) into one DoubleRow contraction with
  per-head zero-padded q operands (zeros memset once, rows rewritten per
  batch).
- exp split across engines: Act does native exp -> fp8 wts; DVE/Pool do
  Schraudolph bf16 exp (int16 bit trick) for a subset of seq-chunks whose
  AV runs in bf16.
- softmax denominator from a ones-column appended to vT (row 64 of the AV
  psum); reciprocal on DVE, broadcast via a tiny PE matmul (ones x recip)
  into PSUM, normalize fused with the PSUM->SBUF move.
- groupnorm stats via bn_stats on bf16 inputs + block-diag matmul
  reduction; rsqrt via quake seed + 2 Newton steps (keeps Act on exp).
- x/y staged in bf16 (halves DMA); residual add reuses the staged x.
"""
import os
import sys

sys.path.insert(0, "/opt/trn_rl_repo")

import numpy as np

import concourse.bacc as bacc
import concourse.bass as bass
import concourse.tile as tile
from concourse import mybir
from concourse.bass_utils import run_bass_kernel_spmd

F32 = mybir.dt.float32
F32R = mybir.dt.float32r
BF16 = mybir.dt.bfloat16
FP8 = mybir.dt.float8e4
I16 = mybir.dt.int16
I32 = mybir.dt.int32
AF = mybir.ActivationFunctionType
OP = mybir.AluOpType
PM = mybir.MatmulPerfMode

B, C, H, W = 16, 512, 32, 32
T = H * W              # 1024
NH = 8                 # heads
CH = C // NH           # 64
GROUPS = 32
GSIZE = C // GROUPS    # 16
EPS = 1e-5
N_CORES = 8
BPC = B // N_CORES     # batches per core
CB = C // 128          # 4 channel blocks
NT = T // 512          # 2 column halves
ST = T // 128          # 8 seq tiles of 128
NK = ST // 2           # 4 DoubleRow seq chunks of 256

WSCALE = 16.0          # fp8 weight pre-scale for Wq/Wk/Wv
QK_SCALE = 1.0 / (WSCALE * WSCALE)   # folded into exp
# Schraudolph bf16 exp: bits16 = x*(2^7/ln2)*QK_SCALE + (127*2^7 - c)
SCH_A = 184.66496580927726 * QK_SCALE
SCH_B = 16250.4

# exp engine per k-chunk (0..3): 'A' = Act native exp (fp8 wts, DR AV),
# 'D'/'P' = DVE/Pool Schraudolph (bf16 wts, bf16 AV). Spill chunks first so
# the head tail drains through the fast Act consumer.
EXP_ENG = ['P', 'D', 'A', 'A']


def _build():
    nc = bacc.Bacc(None, target_bir_lowering=False)

    x2 = nc.dram_tensor("x2", (BPC, C, T), BF16, kind="ExternalInput")
    y2 = nc.dram_tensor("y2", (BPC, C, T), BF16, kind="ExternalInput")
    wq8_d = nc.dram_tensor("wq8", (128, 2, 2, C), FP8, kind="ExternalInput")
    wk8_d = nc.dram_tensor("wk8", (128, 2, 2, C), FP8, kind="ExternalInput")
    wv8_d = nc.dram_tensor("wv8", (128, 2, 2, C), FP8, kind="ExternalInput")
    wpt = nc.dram_tensor("wpt", (C, C), F32, kind="ExternalInput")
    bq_l = nc.dram_tensor("bq_l", (128, CB), F32, kind="ExternalInput")
    bk_l = nc.dram_tensor("bk_l", (128, CB), F32, kind="ExternalInput")
    bp_l = nc.dram_tensor("bp_l", (128, CB), F32, kind="ExternalInput")
    bv_bc = nc.dram_tensor("bv_bc", (128, NH, CH), F32, kind="ExternalInput")
    gnw_l = nc.dram_tensor("gnw_l", (128, CB), F32, kind="ExternalInput")
    gnb_l = nc.dram_tensor("gnb_l", (128, CB), F32, kind="ExternalInput")
    m1 = nc.dram_tensor("m1", (128, 128), F32, kind="ExternalInput")
    vcap8_d = nc.dram_tensor("vcap8", (128, NH, 2), FP8, kind="ExternalInput")
    vcap16_d = nc.dram_tensor("vcap16", (128, NH, 2), BF16, kind="ExternalInput")
    ones64_d = nc.dram_tensor("ones64", (1, CH), F32, kind="ExternalInput")
    out_d = nc.dram_tensor("out", (BPC, C, T), F32, kind="ExternalOutput")

    with tile.TileContext(nc) as tc:
        from contextlib import ExitStack
        with ExitStack() as ctx:
            consts = ctx.enter_context(tc.tile_pool(name="consts", bufs=1))
            px = ctx.enter_context(tc.tile_pool(name="px", bufs=2))
            py = ctx.enter_context(tc.tile_pool(name="py", bufs=2))
            pgn = ctx.enter_context(tc.tile_pool(name="pgn", bufs=4))
            pkq = ctx.enter_context(tc.tile_pool(name="pkq", bufs=4))
            pvt8 = ctx.enter_context(tc.tile_pool(name="pvt8", bufs=4))
            pvt16 = ctx.enter_context(tc.tile_pool(name="pvt16", bufs=8))
            pw8 = ctx.enter_context(tc.tile_pool(name="pw8", bufs=6))
            pw16 = ctx.enter_context(tc.tile_pool(name="pw16", bufs=4))
            pa = ctx.enter_context(tc.tile_pool(name="pa", bufs=2))
            pr0 = ctx.enter_context(tc.tile_pool(name="pr0", bufs=2))
            pst = ctx.enter_context(tc.tile_pool(name="pst", bufs=4))
            post = ctx.enter_context(tc.tile_pool(name="post", bufs=4))
            ps_proj = ctx.enter_context(tc.tile_pool(name="ps_proj", bufs=2, space="PSUM"))
            ps_sc = ctx.enter_context(tc.tile_pool(name="ps_sc", bufs=3, space="PSUM"))
            ps_a = ctx.enter_context(tc.tile_pool(name="ps_a", bufs=1, space="PSUM"))
            ps_aux = ctx.enter_context(tc.tile_pool(name="ps_aux", bufs=1, space="PSUM"))

            # ---------------- constants ----------------
            wq_sb = consts.tile([128, 2, 2, C], FP8, tag="wq")
            wk_sb = consts.tile([128, 2, 2, C], FP8, tag="wk")
            wv_sb = consts.tile([128, 2, 2, C], FP8, tag="wv")
            wp_sb = consts.tile([128, CB, C], F32R, tag="wp")

            def emit_weight_loads():
                nc.sync.dma_start(out=wk_sb, in_=wk8_d[:, :, :, :])
                nc.sync.dma_start(out=wv_sb, in_=wv8_d[:, :, :, :])
                nc.sync.dma_start(out=wq_sb, in_=wq8_d[:, :, :, :])
                nc.sync.dma_start(out=wp_sb, in_=wpt.rearrange("(kb p) o -> p kb o", p=128).bitcast(F32R))

            m1_sb = consts.tile([128, 128], F32, tag="m1")
            bq_sb = consts.tile([128, CB], F32, tag="bq")
            bk_sb = consts.tile([128, CB], F32, tag="bk")
            bp_sb = consts.tile([128, CB], F32, tag="bp")
            bv_sb = consts.tile([128, NH, CH], F32, tag="bv")
            gnw_sb = consts.tile([128, CB], F32, tag="gnw")
            gnb_sb = consts.tile([128, CB], F32, tag="gnb")
            vcap8_sb = consts.tile([128, NH, 2], FP8, tag="vcap8")
            vcap16_sb = consts.tile([128, NH, 2], BF16, tag="vcap16")
            ones64_sb = consts.tile([1, CH], F32, tag="ones64")
            magic_sb = consts.tile([128, CB], I32, tag="magic")
            nc.vector.memset(magic_sb, 0x5f3759df)
            warm = consts.tile([1, 1], F32, tag="warm")
            nc.vector.memset(warm, 0.0)
            nc.scalar.activation(out=warm, in_=warm, func=AF.Exp)

            def emit_small_consts():
                nc.sync.dma_start(out=m1_sb, in_=m1[:, :])
                nc.sync.dma_start(out=gnw_sb, in_=gnw_l[:, :])
                nc.sync.dma_start(out=gnb_sb, in_=gnb_l[:, :])
                nc.sync.dma_start(out=bk_sb, in_=bk_l[:, :])
                nc.sync.dma_start(out=bq_sb, in_=bq_l[:, :])
                nc.sync.dma_start(out=bv_sb, in_=bv_bc[:, :, :])
                nc.sync.dma_start(out=bp_sb, in_=bp_l[:, :])
                nc.sync.dma_start(out=vcap8_sb, in_=vcap8_d[:, :, :])
                nc.sync.dma_start(out=vcap16_sb, in_=vcap16_d[:, :, :])
                nc.sync.dma_start(out=ones64_sb, in_=ones64_d[:, :])

            # persistent zero-padded q operands: [128, 2, T] fp8 per head;
            # head h occupies rows (h%2)*64.. at sub (h//2)%2; zeros persist.
            q_pads = [consts.tile([128, 2, T], FP8, tag=f"qpad{h}", name=f"qpad{h}")
                      for h in range(NH)]

            def emit_qpad_memsets():
                for h, qp in enumerate(q_pads):
                    eng = (nc.gpsimd, nc.vector)[h % 2]
                    eng.memset(qp.rearrange("p a b -> p (a b)"), 0.0)

            # ---------------- groupnorm ----------------
            def groupnorm(src_sb, gn_tiles, spread=False):
                """src_sb: [128, CB, T] bf16. gn_tiles: 2 x [128, 2, T] fp8."""
                mv = pst.tile([128, CB, 2], F32, tag="mv")
                stats6 = pst.tile([128, 2, 6], F32, tag="stats6")
                for cb in range(CB):
                    for c2 in range(2):
                        nc.vector.bn_stats(
                            out=stats6[:, c2, :],
                            in_=src_sb[:, cb, c2 * 512:(c2 + 1) * 512])
                    nc.vector.bn_aggr(out=mv[:, cb, :], in_=stats6)
                musq = pst.tile([128, 4], F32, tag="musq")
                nc.vector.tensor_tensor(out=musq, in0=mv[:, :, 0], in1=mv[:, :, 0], op=OP.mult)
                nc.vector.tensor_tensor(out=mv[:, :, 1], in0=musq, in1=mv[:, :, 1], op=OP.add)
                aux = ps_aux.tile([128, 512], F32, tag="aux")
                psg = aux[:, 0:8]
                nc.tensor.matmul(psg, m1_sb, mv.rearrange("p a b -> p (a b)"), start=True, stop=True)
                gsb = pst.tile([128, 8], F32, tag="gsb")
                nc.vector.tensor_copy(gsb, psg)
                tmp4 = pst.tile([128, 4], F32, tag="tmp4")
                nc.vector.tensor_tensor(out=tmp4, in0=gsb[:, 0::2], in1=gsb[:, 0::2], op=OP.mult)
                vv = pst.tile([128, 4], F32, tag="vv")
                nc.vector.scalar_tensor_tensor(
                    out=vv, in0=gsb[:, 1::2], scalar=EPS, in1=tmp4,
                    op0=OP.add, op1=OP.subtract)
                bsh = pst.tile([128, 4], I32, tag="bsh")
                nc.vector.tensor_scalar(
                    out=bsh, in0=vv.bitcast(I32), scalar1=1, scalar2=None,
                    op0=OP.logical_shift_right)
                nc.vector.tensor_tensor(out=tmp4.bitcast(I32), in0=magic_sb, in1=bsh, op=OP.subtract)
                nrt = pst.tile([128, 4], F32, tag="nrt")
                for _ in range(2):
                    nc.vector.tensor_tensor(out=nrt, in0=tmp4, in1=tmp4, op=OP.mult)
                    nc.vector.scalar_tensor_tensor(
                        out=nrt, in0=nrt, scalar=-0.5, in1=vv, op0=OP.mult, op1=OP.mult)
                    nc.vector.scalar_tensor_tensor(
                        out=tmp4, in0=nrt, scalar=1.5, in1=tmp4, op0=OP.add, op1=OP.mult)
                ab = pst.tile([128, 8], F32, tag="ab")
                nc.vector.tensor_tensor(out=ab[:, 0:4], in0=tmp4, in1=gnw_sb, op=OP.mult)
                tmp4b = pst.tile([128, 4], F32, tag="tmp4b")
                nc.vector.tensor_tensor(out=tmp4b, in0=gsb[:, 0::2], in1=ab[:, 0:4], op=OP.mult)
                nc.vector.tensor_tensor(out=ab[:, 4:8], in0=gnb_sb, in1=tmp4b, op=OP.subtract)
                for cb in range(CB):
                    kb2, i = cb // 2, cb % 2
                    for th in range(NT):
                        eng = nc.vector if not spread else \
                            (nc.vector, nc.gpsimd)[(cb * NT + th) % 2]
                        eng.tensor_scalar(
                            out=gn_tiles[kb2][:, i, th * 512:(th + 1) * 512],
                            in0=src_sb[:, cb, th * 512:(th + 1) * 512],
                            scalar1=ab[:, cb:cb + 1], scalar2=ab[:, 4 + cb:5 + cb],
                            op0=OP.mult, op1=OP.add)

            def emit_load(b, dram, pool, tag):
                sb = pool.tile([128, CB, T], BF16, tag=tag, name=tag)
                for cb in range(CB):
                    nc.sync.dma_start(
                        out=sb[:, cb, :],
                        in_=dram[b].rearrange("(cb p) t -> p cb t", p=128)[:, cb, :])
                return sb

            def emit_loads(b):
                y_sb = emit_load(b, y2, py, "y")
                x_sb = emit_load(b, x2, px, "x")
                return x_sb, y_sb

            def emit_gn(src_sb, tag, spread=False):
                gn_tiles = [pgn.tile([128, 2, T], FP8, tag=tag, name=f"{tag}{kb2}")
                            for kb2 in range(2)]
                groupnorm(src_sb, gn_tiles, spread=spread)
                return gn_tiles

            # ---------------- projections ----------------
            def k_proj(gny):
                kq = [pkq.tile([128, 2, T], FP8, tag="kq", name=f"kq{q}") for q in range(2)]
                for ob in range(CB):
                    quad, i = ob // 2, ob % 2
                    for th in range(NT):
                        psk = ps_proj.tile([128, 512], F32, tag="mm")
                        for kb2 in range(2):
                            nc.tensor.matmul(
                                psk,
                                wk_sb[:, kb2, :, ob * 128:(ob + 1) * 128],
                                gny[kb2][:, :, th * 512:(th + 1) * 512],
                                start=(kb2 == 0), stop=(kb2 == 1),
                                perf_mode=PM.DoubleRow)
                        nc.scalar.activation(
                            out=kq[quad][:, i, th * 512:(th + 1) * 512],
                            in_=psk, func=AF.Identity,
                            bias=bk_sb[:, ob:ob + 1], scale=1.0)
                return kq

            def q_proj(gnx):
                for ob in range(CB):
                    sub = ob % 2
                    h0, h1 = 2 * ob, 2 * ob + 1
                    for th in range(NT):
                        psq = ps_proj.tile([128, 512], F32, tag="mm")
                        for kb2 in range(2):
                            nc.tensor.matmul(
                                psq,
                                wq_sb[:, kb2, :, ob * 128:(ob + 1) * 128],
                                gnx[kb2][:, :, th * 512:(th + 1) * 512],
                                start=(kb2 == 0), stop=(kb2 == 1),
                                perf_mode=PM.DoubleRow)
                        nc.scalar.activation(
                            out=q_pads[h0][0:64, sub, th * 512:(th + 1) * 512],
                            in_=psq[0:64, :], func=AF.Identity,
                            bias=bq_sb[0:64, ob:ob + 1], scale=1.0)
                        nc.scalar.activation(
                            out=q_pads[h1][64:128, sub, th * 512:(th + 1) * 512],
                            in_=psq[64:128, :], func=AF.Identity,
                            bias=bq_sb[64:128, ob:ob + 1], scale=1.0)

            def v_proj(gny):
                vt8 = {k: pvt8.tile([128, 2, NH, CH + 2], FP8, tag="vt8", name=f"vt8_{k}")
                       for k in range(NK) if EXP_ENG[k] == 'A'}
                vt16 = {st: pvt16.tile([128, NH, CH + 2], BF16, tag="vt16", name=f"vt16_{st}")
                        for st in range(ST) if EXP_ENG[st // 2] != 'A'}
                for tt in range(ST):
                    psv = ps_proj.tile([128, 512], F32, tag="mm")
                    for kb2 in range(2):
                        nc.tensor.matmul(
                            psv,
                            gny[kb2][:, :, tt * 128:(tt + 1) * 128],
                            wv_sb[:, kb2, :, :],
                            start=(kb2 == 0), stop=(kb2 == 1),
                            perf_mode=PM.DoubleRow)
                    if EXP_ENG[tt // 2] == 'A':
                        dst = vt8[tt // 2]
                        nc.vector.tensor_tensor(
                            out=dst[:, tt % 2, :, 0:CH],
                            in0=psv.rearrange("p (h c) -> p h c", h=NH),
                            in1=bv_sb, op=OP.add)
                        nc.vector.tensor_copy(dst[:, tt % 2, :, CH:CH + 2], vcap8_sb)
                    else:
                        dst = vt16[tt]
                        nc.vector.tensor_tensor(
                            out=dst[:, :, 0:CH],
                            in0=psv.rearrange("p (h c) -> p h c", h=NH),
                            in1=bv_sb, op=OP.add)
                        nc.vector.tensor_copy(dst[:, :, CH:CH + 2], vcap16_sb)
                return vt8, vt16

            # ---------------- attention ----------------
            def attention_head(bctx, b, h, a_sb):
                kq = bctx["kq"]
                vt8, vt16 = bctx["vt"]
                quad = h // 4
                psa = ps_a.tile([CH + 2, T], F32, tag="psa")
                last_k = NK - 1
                wtiles = {}
                for k in range(NK):
                    if EXP_ENG[k] == 'A':
                        wtiles[k] = pw8.tile([128, 2, T], FP8, tag="w8", name=f"w8_{h}_{k}")
                    else:
                        wtiles[k] = pw16.tile([128, 2, T], BF16, tag="w16", name=f"w16_{h}_{k}")
                # QK + exp, engine-alternating (k fastest) for pipeline depth
                for stp in range(2):
                    for th in range(NT):
                        for k in range(NK):
                            st = 2 * k + stp
                            eng = EXP_ENG[k]
                            wts = wtiles[k]
                            scores = ps_sc.tile([128, 512], F32, tag="sc")
                            nc.tensor.matmul(
                                scores,
                                kq[quad][:, :, st * 128:(st + 1) * 128],
                                q_pads[h][:, :, th * 512:(th + 1) * 512],
                                start=True, stop=True,
                                perf_mode=PM.DoubleRow)
                            if eng == 'A':
                                nc.scalar.activation(
                                    out=wts[:, stp, th * 512:(th + 1) * 512],
                                    in_=scores, func=AF.Exp, scale=QK_SCALE)
                            else:
                                veng = nc.vector if eng == 'D' else nc.gpsimd
                                veng.tensor_scalar(
                                    out=wts[:, stp, th * 512:(th + 1) * 512].bitcast(I16),
                                    in0=scores, scalar1=SCH_A, scalar2=SCH_B,
                                    op0=OP.mult, op1=OP.add)
                # AV burst (accumulate into psa)
                for k in range(NK):
                    wts = wtiles[k]
                    if EXP_ENG[k] == 'A':
                        for th in range(NT):
                            nc.tensor.matmul(
                                psa[:, th * 512:(th + 1) * 512],
                                vt8[k][:, :, h, :],
                                wts[:, :, th * 512:(th + 1) * 512],
                                start=(k == 0), stop=(k == last_k),
                                perf_mode=PM.DoubleRow,
                                skip_group_check=True)
                    else:
                        for stp in range(2):
                            st = 2 * k + stp
                            for th in range(NT):
                                nc.tensor.matmul(
                                    psa[:, th * 512:(th + 1) * 512],
                                    vt16[st][:, h, :],
                                    wts[:, stp, th * 512:(th + 1) * 512],
                                    start=(k == 0 and stp == 0),
                                    stop=(k == last_k and stp == 1),
                                    skip_group_check=True)
                # ---- softmax denominator + normalize into a_sb ----
                rows = slice((h % 2) * 64, (h % 2) * 64 + 64)
                cbh = h // 2
                r0r = pr0.tile([1, T], F32, tag="r0")
                for th in range(NT):
                    nc.vector.reciprocal_approx_fast(
                        out=r0r[:, th * 512:(th + 1) * 512],
                        in_=psa[CH:CH + 1, th * 512:(th + 1) * 512])
                    aux = ps_aux.tile([128, 512], F32, tag="aux")
                    psb = aux[0:64, :]
                    nc.tensor.matmul(
                        psb, ones64_sb.bitcast(F32R),
                        r0r[:, th * 512:(th + 1) * 512].bitcast(F32R),
                        start=True, stop=True)
                    eng = nc.vector if th == 0 else nc.gpsimd
                    eng.tensor_tensor(
                        out=a_sb[rows, cbh, th * 512:(th + 1) * 512],
                        in0=psa[0:CH, th * 512:(th + 1) * 512],
                        in1=psb, op=OP.mult)

            # ---------------- output projection ----------------
            def p_proj_ob(b, a_sb, x_sb, ob):
                for th in range(NT):
                    psh = ps_proj.tile([128, 512], F32, tag="mm")
                    for kb in range(CB):
                        nc.tensor.matmul(
                            psh,
                            wp_sb[:, kb, ob * 128:(ob + 1) * 128],
                            a_sb[:, kb, th * 512:(th + 1) * 512],
                            start=(kb == 0), stop=(kb == CB - 1))
                    ost = post.tile([128, 512], F32, tag="ost")
                    eng = nc.gpsimd if th % 2 == 0 else nc.vector
                    eng.scalar_tensor_tensor(
                        out=ost, in0=psh, scalar=bp_sb[:, ob:ob + 1],
                        in1=x_sb[:, ob, th * 512:(th + 1) * 512],
                        op0=OP.add, op1=OP.add)
                    nc.sync.dma_start(
                        out=out_d[b].rearrange("(cb p) t -> p cb t", p=128)[:, ob, th * 512:(th + 1) * 512],
                        in_=ost)

            # ---------------- batch pipeline ----------------
            bctxs = [dict() for _ in range(BPC)]
            y0 = emit_load(0, y2, py, "y")
            emit_small_consts()
            x0 = emit_load(0, x2, px, "x")
            bctxs[0]["x"] = x0
            emit_weight_loads()
            with tc.high_priority(10**6):
                bctxs[0]["gny"] = emit_gn(y0, "gny", spread=True)
                bctxs[0]["gnx"] = emit_gn(x0, "gnx", spread=True)
            emit_qpad_memsets()
            with tc.high_priority(10**6):
                bctxs[0]["kq"] = k_proj(bctxs[0]["gny"])
                bctxs[0]["vt"] = v_proj(bctxs[0]["gny"])
                q_proj(bctxs[0]["gnx"])

            prev = None  # (b, a_sb, x_sb) of the previous batch, p-proj pending
            for b in range(BPC):
                bctx = bctxs[b]
                nb = bctxs[b + 1] if b + 1 < BPC else None
                a_sb = pa.tile([128, CB, T], F32R, tag="a")
                for h in range(NH):
                    # attention stream outranks injected background work in
                    # the scheduler's priority heap (relative order preserved)
                    with tc.high_priority(10**6):
                        attention_head(bctx, b, h, a_sb)
                    if prev is not None and h < CB:
                        p_proj_ob(prev[0], prev[1], prev[2], h)
                        if h == CB - 1:
                            prev = None
                    if nb is not None:
                        if h == 0:
                            nxy = emit_loads(b + 1)
                            nb["x"] = nxy[0]
                            nb["_y"] = nxy[1]
                        elif h == 1:
                            nb["gny"] = emit_gn(nb["_y"], "gny")
                        elif h == 2:
                            nb["gnx"] = emit_gn(nb["x"], "gnx")
                        elif h == 3:
                            nb["kq"] = k_proj(nb["gny"])
                        elif h == 4:
                            nb["vt"] = v_proj(nb["gny"])
                        elif h == 5:
                            q_proj(nb["gnx"])
                prev = (b, a_sb, bctx["x"])
            for ob in range(CB):
                p_proj_ob(prev[0], prev[1], prev[2], ob)

    nc.finalize()
    return nc


_NC = None


def _get_nc():
    global _NC
    if _NC is None:
        _NC = _build()
    return _NC


def _prep_inputs(x, y, gn_w, gn_b, Wq, bq, Wkv, bkv, Wp, bp):
    import ml_dtypes
    FP8NP = ml_dtypes.float8_e4m3fn
    scale = CH ** -0.25
    idx_k = np.concatenate([np.arange(h * 2 * CH, h * 2 * CH + CH) for h in range(NH)])
    idx_v = np.concatenate([np.arange(h * 2 * CH + CH, (h + 1) * 2 * CH) for h in range(NH)])

    def dr_layout(wt):  # [C_in, C_out] -> [128, 2, 2, C_out]
        return np.ascontiguousarray(wt.reshape(2, 2, 128, C).transpose(2, 0, 1, 3))

    wq8 = dr_layout((Wq * (scale * WSCALE)).T).astype(FP8NP)
    wk8 = dr_layout((Wkv[idx_k] * (scale * WSCALE)).T).astype(FP8NP)
    wv8 = dr_layout((Wkv[idx_v] * WSCALE).T).astype(FP8NP)
    wpt = np.ascontiguousarray(Wp.T / WSCALE).astype(np.float32)

    def part_layout(v):
        return np.ascontiguousarray(v.reshape(CB, 128).T)

    bq_l = part_layout(bq * (scale * WSCALE))
    bk_l = part_layout(bkv[idx_k] * (scale * WSCALE))
    bp_l = part_layout(bp)
    gnw_l = part_layout(gn_w)
    gnb_l = part_layout(gn_b)
    bv = bkv[idx_v] * WSCALE
    bv_bc = np.broadcast_to(bv.reshape(1, NH, CH), (128, NH, CH)).copy().astype(np.float32)
    m1 = np.zeros((128, 128), np.float32)
    for g in range(128 // GSIZE):
        m1[g * GSIZE:(g + 1) * GSIZE, g * GSIZE:(g + 1) * GSIZE] = 1.0 / GSIZE
    vcap = np.zeros((128, NH, 2), np.float32)
    vcap[:, :, 0] = 1.0
    vcap8 = vcap.astype(FP8NP)
    vcap16 = vcap.astype(ml_dtypes.bfloat16)
    ones64 = np.ones((1, CH), np.float32)

    xf = x.reshape(B, C, T).astype(ml_dtypes.bfloat16)
    yf = y.reshape(B, C, T).astype(ml_dtypes.bfloat16)

    shared = {
        "wq8": wq8, "wk8": wk8, "wv8": wv8, "wpt": wpt,
        "bq_l": bq_l, "bk_l": bk_l, "bp_l": bp_l, "bv_bc": bv_bc,
        "gnw_l": gnw_l, "gnb_l": gnb_l, "m1": m1,
        "vcap8": vcap8, "vcap16": vcap16, "ones64": ones64,
    }
    in_maps = []
    for i in range(N_CORES):
        m = dict(shared)
        m["x2"] = np.ascontiguousarray(xf[i * BPC:(i + 1) * BPC])
        m["y2"] = np.ascontiguousarray(yf[i * BPC:(i + 1) * BPC])
        in_maps.append(m)
    return in_maps


def kernel(x, y, gn_w, gn_b, Wq, bq, Wkv, bkv, Wp, bp):
    args = [np.asarray(a, dtype=np.float32) for a in
            (x, y, gn_w, gn_b, Wq, bq, Wkv, bkv, Wp, bp)]
    in_maps = _prep_inputs(*args)
    nc = _get_nc()
    res = run_bass_kernel_spmd(nc, in_maps, core_ids=list(range(N_CORES)))
    out = np.empty((B, C, T), np.float32)
    for i in range(N_CORES):
        out[i * BPC:(i + 1) * BPC] = res.results[i]["out"]
    return out.reshape(B, C, H, W)
